# revision 1
# baseline (speedup 1.0000x reference)
"""Trainium2 Bass kernel for a dense transformer block (pre-LN, MHA + MLP).

Full inputs in, full outputs out. Sharding: 8 cores = (batch, seq-half).
Each core computes K/V over its batch element's full 1024 tokens and
Q/attention/MLP over its own 512 tokens (host permutes tokens so the core's
own half is always rows 0..511 — softmax over keys is permutation invariant).
No collectives needed.

Host-side preprocessing folds LayerNorm affine params into the following
matmul weights:  (xhat*g + b) @ W == xhat @ (diag(g) W) + b @ W.

All large on-chip buffers are split into per-slice tiles because the Tile
scheduler tracks dependencies at tile granularity — fine tiles let
consumers start as soon as their slice is ready.
"""

import sys

sys.path.insert(0, "/opt/trn_rl_repo")

import numpy as np

import concourse.bass as bass
import concourse.bacc as bacc
import concourse.mybir as mybir
import concourse.tile as tile
from concourse.bass_utils import run_bass_kernel_spmd
from concourse.masks import make_identity

P = 128
C = 1024
HEADS = 16
DH = 64
HID = 4096
NTOK = 1024  # tokens per batch element (kv length)
NOWN = 512  # tokens owned by this core (q length)
SCALE = DH ** -0.5
EPS = 1e-5

F32 = mybir.dt.float32
F32R = mybir.dt.float32r
BF16 = mybir.dt.bfloat16
AF = mybir.ActivationFunctionType
OP = mybir.AluOpType

CT = C // P  # 8 column tiles of the model dim
TT = NTOK // P  # 8 token tiles (kv)
QT = NOWN // P  # 4 token tiles (own)
HT = HID // P  # 32 hidden tiles

DEBUG_DUMPS = False  # set True to add per-phase debug outputs


def build_program():
    nc = bacc.Bacc("TRN2", target_bir_lowering=False)

    io = {}
    io["x"] = nc.dram_tensor("x", (NTOK, C), F32, kind="ExternalInput")
    io["qw"] = nc.dram_tensor("qw", (C, C), F32R, kind="ExternalInput")
    io["kw"] = nc.dram_tensor("kw", (C, C), F32R, kind="ExternalInput")
    io["vw"] = nc.dram_tensor("vw", (C, C), F32R, kind="ExternalInput")
    io["pw"] = nc.dram_tensor("pw", (C, C), F32R, kind="ExternalInput")
    io["f1w"] = nc.dram_tensor("f1w", (C, HID), F32R, kind="ExternalInput")
    io["f2w"] = nc.dram_tensor("f2w", (HID, C), BF16, kind="ExternalInput")
    # biases pre-transposed on host into [128, n] per-partition layout
    io["qbt"] = nc.dram_tensor("qbt", (P, CT), F32, kind="ExternalInput")
    io["kbt"] = nc.dram_tensor("kbt", (P, CT), F32, kind="ExternalInput")
    io["vbt"] = nc.dram_tensor("vbt", (P, CT), F32, kind="ExternalInput")
    io["f1bt"] = nc.dram_tensor("f1bt", (P, HT), F32, kind="ExternalInput")
    io["pb"] = nc.dram_tensor("pb", (C,), F32, kind="ExternalInput")
    io["f2b"] = nc.dram_tensor("f2b", (C,), F32, kind="ExternalInput")
    io["out"] = nc.dram_tensor("out", (NOWN, C), F32, kind="ExternalOutput")
    if DEBUG_DUMPS:
        io["d_hT"] = nc.dram_tensor(
            "d_hT", (P, CT, NTOK), F32, kind="ExternalOutput"
        )
        io["d_kT"] = nc.dram_tensor(
            "d_kT", (P, CT, NTOK), F32, kind="ExternalOutput"
        )
        io["d_qT"] = nc.dram_tensor(
            "d_qT", (P, CT, NOWN), F32, kind="ExternalOutput"
        )
        io["d_vh"] = nc.dram_tensor(
            "d_vh", (P, HEADS, TT, DH + 1), BF16, kind="ExternalOutput"
        )
        io["d_oT"] = nc.dram_tensor(
            "d_oT", (P, CT, NOWN), F32, kind="ExternalOutput"
        )
        io["d_x2"] = nc.dram_tensor(
            "d_x2", (P, QT, C), F32, kind="ExternalOutput"
        )
        io["d_h2T"] = nc.dram_tensor(
            "d_h2T", (P, CT, NOWN), F32, kind="ExternalOutput"
        )
        io["d_actT"] = nc.dram_tensor(
            "d_actT", (P, HT, NOWN), BF16, kind="ExternalOutput"
        )

    with tile.TileContext(nc) as tc:
        _emit(nc, tc, io)
    nc.compile()
    return nc


def _emit(nc, tc, io):
    x_d, out_d = io["x"], io["out"]

    with (
        tc.tile_pool(name="consts", bufs=1) as consts,
        tc.tile_pool(name="persist", bufs=1) as persist,
        tc.tile_pool(name="big", bufs=1) as big,
        tc.tile_pool(name="psum_tr", bufs=2, space="PSUM") as psum_tr,
    ):
        # ---- constants ----
        ident_f32 = consts.tile([P, P], F32)
        make_identity(nc, ident_f32)
        ident = consts.tile([P, P], F32R)
        nc.vector.tensor_copy(out=ident, in_=ident_f32)
        eps_tile = consts.tile([P, 1], F32)
        nc.vector.memset(eps_tile, EPS)
        qbT = consts.tile([P, CT], F32)
        nc.sync.dma_start(qbT, io["qbt"][:, :])
        kbT = consts.tile([P, CT], F32)
        nc.sync.dma_start(kbT, io["kbt"][:, :])
        vbT = consts.tile([P, CT], F32)
        nc.sync.dma_start(vbT, io["vbt"][:, :])
        f1bT = consts.tile([P, HT], F32)
        nc.sync.dma_start(f1bT, io["f1bt"][:, :])

        def bcast_const(src_d, n):
            t = consts.tile([P, n], F32)
            src = bass.AP(tensor=src_d, offset=0, ap=[[0, P], [1, n]])
            nc.sync.dma_start(t, src)
            return t

        pb_bc = bcast_const(io["pb"], C)
        f2b_bc = bcast_const(io["f2b"], C)

        # own x tiles (fp32, kept for the residual), one tile per token tile
        x_own = []
        for t in range(QT):
            xo = persist.tile([P, C], F32, tag=f"xo{t}", name=f"xo{t}")
            nc.sync.dma_start(xo, x_d[t * P : (t + 1) * P, :])
            x_own.append(xo)
        x2 = [
            persist.tile([P, C], F32, tag=f"x2_{t}", name=f"x2_{t}")
            for t in range(QT)
        ]

        def layernorm_tile(temps, xt):
            """xt: [128, C] fp32 -> returns normalized f32r tile [128, C]."""
            stats = temps.tile([P, 2, 6], F32, tag="ln_stats", name="st")
            for sg in range(2):
                nc.vector.bn_stats(
                    out=stats[:, sg, :], in_=xt[:, sg * 512 : (sg + 1) * 512]
                )
            mv = temps.tile([P, 2], F32, tag="ln_mv", name="mv")
            nc.vector.bn_aggr(out=mv[:], in_=stats[:])
            rstd = temps.tile([P, 1], F32, tag="ln_rstd", name="rstd")
            nc.scalar.activation(
                out=rstd, in_=mv[:, 1:2], func=AF.Sqrt, bias=eps_tile, scale=1.0
            )
            nc.vector.reciprocal(out=rstd, in_=rstd)
            nmr = temps.tile([P, 1], F32, tag="ln_nmr", name="nmr")
            nc.vector.tensor_tensor(nmr, mv[:, 0:1], rstd, OP.mult)
            nc.vector.tensor_scalar_mul(nmr, nmr, -1.0)
            h = temps.tile([P, C], F32R, tag="ln_h", name="h")
            nc.vector.tensor_scalar(
                out=h,
                in0=xt,
                scalar1=rstd,
                scalar2=nmr,
                op0=OP.mult,
                op1=OP.add,
            )
            return h

        def transpose_into(dst_view, src_view):
            """dst_view [128, 128] (f32r) <- transpose of src_view."""
            ps = psum_tr.tile([P, P], F32R, tag="tr", name="tr")
            nc.tensor.transpose(ps, src_view, ident)
            nc.any.tensor_copy(out=dst_view, in_=ps)

        # ---- per-slice phase buffers (tag-shared slots, serial reuse) ----
        # hT[(c, t2)]: [P, 512] f32r; slots reused later by actT (tag A*)
        hT = {
            (c, t2): big.tile(
                [P, 512], F32R, tag=f"A{(c * 2 + t2) % 16}", name=f"hT{c}_{t2}"
            )
            for c in range(CT)
            for t2 in range(2)
        }
        # kT[(ft, t2)]: [P, 512] f32r; slots reused later by h2T (tag B*)
        kT = {
            (ft, t2): big.tile(
                [P, 512], BF16, tag=f"B{(ft * 2 + t2) % 16}", name=f"kT{ft}_{t2}"
            )
            for ft in range(CT)
            for t2 in range(2)
        }
        # vh[h]: [P, TT, DH+1] bf16 head-padded V; V-tags reused by f2w groups
        vh = [
            big.tile([P, TT, P], BF16, tag=f"V{h}", name=f"vh{h}")
            for h in range(HEADS)
        ]
        qT = [
            big.tile([P, 512], BF16, tag=f"D{ft}", name=f"qT{ft}")
            for ft in range(CT)
        ]
        oT = [
            big.tile([P, 512], F32R, tag=f"E{ft}", name=f"oT{ft}")
            for ft in range(CT)
        ]

        # ================= Phase 1: LN1 -> hT =================
        with (
            tc.tile_pool(name="ln1", bufs=3) as ln1,
            tc.tile_pool(name="xtmp", bufs=3) as xtmp,
        ):
            for t in range(TT):
                if t < QT:
                    xt = x_own[t]
                else:
                    xt = xtmp.tile([P, C], F32, tag="xt", name="xt")
                    nc.sync.dma_start(xt, x_d[t * P : (t + 1) * P, :])
                h = layernorm_tile(ln1, xt)
                t2, tb = t // QT, t % QT
                for ft in range(CT):
                    transpose_into(
                        hT[(ft, t2)][:, tb * P : (tb + 1) * P],
                        h[:, ft * P : (ft + 1) * P],
                    )

        # ================= Phase 2: QKV =================
        for h in range(HEADS):
            nc.vector.memset(vh[h][:], 0.0)
            nc.vector.memset(vh[h][:, :, DH : DH + 1], 1.0)
        with (
            tc.tile_pool(name="wchunk", bufs=4) as wpool,
            tc.tile_pool(name="qkv_psum", bufs=4, space="PSUM") as qkv_psum,
        ):

            def kv_sweep(t2, w_d, bT, which):
                for ft in range(CT):
                    slab = wpool.tile([P, CT, P], F32R, tag="w_kv", name="slab")
                    nc.sync.dma_start(
                        slab,
                        w_d[:, ft * P : (ft + 1) * P].rearrange(
                            "(c p) f -> p c f", p=P
                        ),
                    )
                    ps = qkv_psum.tile([P, 512], F32, tag="kvps", name="kvps")
                    for c in range(CT):
                        nc.tensor.matmul(
                            ps,
                            lhsT=slab[:, c, :],
                            rhs=hT[(c, t2)],
                            start=(c == 0),
                            stop=(c == CT - 1),
                        )
                    if which == "k":
                        nc.vector.tensor_scalar(
                            out=kT[(ft, t2)],
                            in0=ps,
                            scalar1=bT[:, ft : ft + 1],
                            scalar2=None,
                            op0=OP.add,
                        )
                    else:
                        # vT tile [feat128, tok512] + bias; transpose 128x128
                        # blocks into head-padded vh layout.
                        vt = wpool.tile([P, 512], F32R, tag="vt_sb", name="vt")
                        nc.vector.tensor_scalar(
                            out=vt,
                            in0=ps,
                            scalar1=bT[:, ft : ft + 1],
                            scalar2=None,
                            op0=OP.add,
                        )
                        for b in range(4):
                            t = t2 * 4 + b
                            ps2 = psum_tr.tile([P, P], F32R, tag="tr", name="tr")
                            nc.tensor.transpose(
                                ps2, vt[:, b * P : (b + 1) * P], ident
                            )
                            # feat rows ft*128..: heads 2*ft, 2*ft+1
                            for hh in range(2):
                                nc.any.tensor_copy(
                                    out=vh[2 * ft + hh][:, t, :DH],
                                    in_=ps2[:, hh * DH : (hh + 1) * DH],
                                )

            # token half 0 only needs LN of tiles 0-3; K/V half 0 and Q can
            # overlap with LN of tiles 4-7.
            kv_sweep(0, io["kw"], kbT, "k")
            kv_sweep(0, io["vw"], vbT, "v")
            for ft in range(CT):
                slab = wpool.tile([P, CT, P], F32R, tag="w_kv", name="qslab")
                nc.sync.dma_start(
                    slab,
                    io["qw"][:, ft * P : (ft + 1) * P].rearrange(
                        "(c p) f -> p c f", p=P
                    ),
                )
                ps = qkv_psum.tile([P, 512], F32, tag="kvps", name="qps")
                for c in range(CT):
                    nc.tensor.matmul(
                        ps,
                        lhsT=slab[:, c, :],
                        rhs=hT[(c, 0)],
                        start=(c == 0),
                        stop=(c == CT - 1),
                    )
                nc.vector.tensor_scalar(
                    out=qT[ft],
                    in0=ps,
                    scalar1=qbT[:, ft : ft + 1],
                    scalar2=None,
                    op0=OP.add,
                )
            kv_sweep(1, io["kw"], kbT, "k")
            kv_sweep(1, io["vw"], vbT, "v")

        if DEBUG_DUMPS:
            for (c, t2), t_ in hT.items():
                nc.sync.dma_start(
                    io["d_hT"][:, c, t2 * 512 : (t2 + 1) * 512],
                    t_[:].bitcast(F32),
                )
            for (ft, t2), t_ in kT.items():
                nc.sync.dma_start(
                    io["d_kT"][:, ft, t2 * 512 : (t2 + 1) * 512],
                    t_[:].bitcast(F32),
                )
            for ft in range(CT):
                nc.sync.dma_start(io["d_qT"][:, ft, :], qT[ft][:].bitcast(F32))
            for h in range(HEADS):
                nc.sync.dma_start(io["d_vh"][:, h, :, :], vh[h][:])

        # ================= Phase 3: attention =================
        with (
            tc.tile_pool(name="attn", bufs=2) as attn_pool,
            tc.tile_pool(name="attn_st", bufs=3, space="PSUM") as attn_st,
            tc.tile_pool(name="attn_ot", bufs=2, space="PSUM") as attn_ot,
        ):
            for h in range(HEADS):
                prow = (h % 2) * DH
                ftile = h // 2
                p_sb = attn_pool.tile([P, TT, NOWN], BF16, tag="p_sb", name="p")
                for c in range(TT):
                    kv_slice = kT[(ftile, c // 4)][
                        prow : prow + DH, (c % 4) * P : (c % 4 + 1) * P
                    ]
                    st = attn_st.tile([P, 512], F32, tag="st", name="st")
                    nc.tensor.matmul(
                        st,
                        lhsT=kv_slice,
                        rhs=qT[ftile][prow : prow + DH, :],
                        start=True,
                        stop=True,
                    )
                    # p = exp(SCALE * s)   (bf16 out)
                    nc.scalar.activation(
                        out=p_sb[:, c, :], in_=st, func=AF.Exp, scale=SCALE
                    )
                ot = attn_ot.tile([P, 512], F32, tag="ot", name="ot")
                for c in range(TT):
                    nc.tensor.matmul(
                        ot,
                        lhsT=vh[h][:, c, :],
                        rhs=p_sb[:, c, :],
                        start=(c == 0),
                        stop=(c == TT - 1),
                    )
                # softmax denominators arrive in row DH (ones column of vh)
                rs = attn_pool.tile([1, NOWN], F32, tag="rs", name="rs")
                nc.vector.reciprocal(out=rs, in_=ot[DH : DH + 1, :])
                rsb = attn_pool.tile([DH, NOWN], F32, tag="rsb", name="rsb")
                nc.gpsimd.partition_broadcast(rsb, rs)
                nc.vector.tensor_tensor(
                    oT[ftile][prow : prow + DH, :], ot[:DH, :], rsb, OP.mult
                )

        if DEBUG_DUMPS:
            for ft in range(CT):
                nc.sync.dma_start(io["d_oT"][:, ft, :], oT[ft][:].bitcast(F32))

        # ================= Phase 4: proj + residual -> x2 =================
        with (
            tc.tile_pool(name="pwc", bufs=2) as pwc,
            tc.tile_pool(name="proj_ps", bufs=1, space="PSUM") as proj_ps,
        ):
            for ns in range(2):
                nsl = slice(ns * 512, (ns + 1) * 512)
                pss = [
                    proj_ps.tile([P, 512], F32, tag=f"pps{tq}", name=f"pps{tq}")
                    for tq in range(QT)
                ]
                for fh in range(2):
                    slab = pwc.tile([P, 4, 512], F32R, tag="pw", name="pwslab")
                    nc.sync.dma_start(
                        slab,
                        io["pw"][fh * 512 : (fh + 1) * 512, nsl].rearrange(
                            "(c p) n -> p c n", p=P
                        ),
                    )
                    for c in range(4):
                        f = fh * 4 + c
                        for tq in range(QT):
                            nc.tensor.matmul(
                                pss[tq],
                                lhsT=oT[f][:, tq * P : (tq + 1) * P],
                                rhs=slab[:, c, :],
                                start=(f == 0),
                                stop=(f == CT - 1),
                            )
                for tq in range(QT):
                    nc.vector.tensor_add(pss[tq], pss[tq], pb_bc[:, nsl])
                    nc.vector.tensor_add(
                        x2[tq][:, nsl], pss[tq], x_own[tq][:, nsl]
                    )

        if DEBUG_DUMPS:
            for tq in range(QT):
                nc.sync.dma_start(io["d_x2"][:, tq, :], x2[tq][:])

        # ================= Phase 5: LN2 -> h2T (reuses kT slots) ==========
        h2T = [
            big.tile([P, 512], F32R, tag=f"B{c}", name=f"h2T{c}")
            for c in range(CT)
        ]
        with tc.tile_pool(name="ln2", bufs=3) as ln2:
            for t in range(QT):
                h = layernorm_tile(ln2, x2[t])
                for ft in range(CT):
                    transpose_into(
                        h2T[ft][:, t * P : (t + 1) * P],
                        h[:, ft * P : (ft + 1) * P],
                    )

        if DEBUG_DUMPS:
            for ft in range(CT):
                nc.sync.dma_start(
                    io["d_h2T"][:, ft, :], h2T[ft][:].bitcast(F32)
                )

        # ================= Phase 6: FC1 + gelu -> actT (reuses hT slots) ==
        def _act_tag(hf):
            if hf < 16:
                return f"A{hf}"
            if hf < 24:
                return f"D{hf - 16}"
            return f"E{hf - 24}"

        actT = [
            big.tile([P, 512], BF16, tag=_act_tag(hf), name=f"actT{hf}")
            for hf in range(HT)
        ]
        with (
            tc.tile_pool(name="f1c", bufs=4) as f1c,
            tc.tile_pool(name="f1_ps", bufs=4, space="PSUM") as f1_ps,
        ):
            for hf in range(HT):
                ps = f1_ps.tile([P, 512], F32, tag="f1ps", name="f1ps")
                slab = f1c.tile([P, CT, P], F32R, tag="f1w", name="f1slab")
                nc.sync.dma_start(
                    slab,
                    io["f1w"][:, hf * P : (hf + 1) * P].rearrange(
                        "(c p) f -> p c f", p=P
                    ),
                )
                for c in range(CT):
                    nc.tensor.matmul(
                        ps,
                        lhsT=slab[:, c, :],
                        rhs=h2T[c],
                        start=(c == 0),
                        stop=(c == CT - 1),
                    )
                # gelu(ps + f1b), fused bias via activation
                nc.scalar.activation(
                    out=actT[hf],
                    in_=ps,
                    func=AF.Gelu,
                    bias=f1bT[:, hf : hf + 1],
                    scale=1.0,
                )

        if DEBUG_DUMPS:
            for hf in range(HT):
                nc.sync.dma_start(io["d_actT"][:, hf, :], actT[hf][:])

        # ================= Phase 7: FC2 + residual -> out =================
        # f2w streamed in 512KB groups of 4 hidden-tiles (reuses vh V-tags)
        with (
            tc.tile_pool(name="f2_ps", bufs=4, space="PSUM") as f2_ps,
            tc.tile_pool(name="out_sb", bufs=2) as out_pool,
        ):
            for ns in range(2):
                nsl = slice(ns * 512, (ns + 1) * 512)
                groups = []
                for g in range(8):
                    gw = big.tile(
                        [P, 4, 512], BF16, tag=f"V{g}", name=f"f2wg{g}_{ns}"
                    )
                    nc.sync.dma_start(
                        gw,
                        io["f2w"][g * 512 : (g + 1) * 512, nsl].rearrange(
                            "(o p) n -> p o n", p=P
                        ),
                    )
                    groups.append(gw)
                for tq in range(QT):
                    ps = f2_ps.tile([P, 512], F32, tag="f2ps", name="f2ps")
                    for hc in range(HT):
                        nc.tensor.matmul(
                            ps,
                            lhsT=actT[hc][:, tq * P : (tq + 1) * P],
                            rhs=groups[hc // 4][:, hc % 4, :],
                            start=(hc == 0),
                            stop=(hc == HT - 1),
                        )
                    ot2 = out_pool.tile([P, 512], F32, tag="out_t", name="o")
                    nc.vector.tensor_add(ps, ps, f2b_bc[:, nsl])
                    nc.vector.tensor_add(ot2, ps, x2[tq][:, nsl])
                    nc.sync.dma_start(out_d[tq * P : (tq + 1) * P, nsl], ot2)


_PROGRAM = None


def _get_program():
    global _PROGRAM
    if _PROGRAM is None:
        _PROGRAM = build_program()
    return _PROGRAM


def build_in_maps(inputs):
    x = np.asarray(inputs["x"], np.float32)  # [4, 1024, 1024]
    ln1_g = np.asarray(inputs["ln1_g"], np.float64)
    ln1_b = np.asarray(inputs["ln1_b"], np.float64)
    ln2_g = np.asarray(inputs["ln2_g"], np.float64)
    ln2_b = np.asarray(inputs["ln2_b"], np.float64)
    qkv_w = np.asarray(inputs["qkv_w"], np.float64)
    qkv_b = np.asarray(inputs["qkv_b"], np.float64)
    proj_w = np.asarray(inputs["proj_w"], np.float32)
    proj_b = np.asarray(inputs["proj_b"], np.float32)
    fc1_w = np.asarray(inputs["fc1_w"], np.float64)
    fc1_b = np.asarray(inputs["fc1_b"], np.float64)
    fc2_w = np.asarray(inputs["fc2_w"], np.float32)
    fc2_b = np.asarray(inputs["fc2_b"], np.float32)

    # Fold LN affine into the following matmul:
    #   (xhat*g + b) @ W == xhat @ (diag(g) W) + b @ W
    qkv_w_f = (ln1_g[:, None] * qkv_w).astype(np.float32)
    qkv_b_f = (qkv_b + ln1_b @ qkv_w).astype(np.float32)
    f1w_f = (ln2_g[:, None] * fc1_w).astype(np.float32)
    f1b_f = (fc1_b + ln2_b @ fc1_w).astype(np.float32)

    qw = np.ascontiguousarray(qkv_w_f[:, :C])
    kw = np.ascontiguousarray(qkv_w_f[:, C : 2 * C])
    vw = np.ascontiguousarray(qkv_w_f[:, 2 * C :])

    def tbias(b):  # [n*128] -> [128, n] per-partition layout
        return np.ascontiguousarray(b.reshape(-1, P).T)

    import ml_dtypes

    f2w_bf = fc2_w.astype(ml_dtypes.bfloat16)

    common = dict(
        qw=qw, kw=kw, vw=vw, pw=proj_w, f1w=f1w_f, f2w=f2w_bf,
        qbt=tbias(qkv_b_f[:C]),
        kbt=tbias(qkv_b_f[C : 2 * C]),
        vbt=tbias(qkv_b_f[2 * C :]),
        f1bt=tbias(f1b_f),
        pb=proj_b, f2b=fc2_b,
    )
    in_maps = []
    for core in range(8):
        b, half = core // 2, core % 2
        own = x[b, half * NOWN : (half + 1) * NOWN, :]
        other = x[b, (1 - half) * NOWN : (2 - half) * NOWN, :]
        xp = np.ascontiguousarray(np.concatenate([own, other], axis=0))
        in_maps.append({**common, "x": xp})
    return in_maps


def kernel(**inputs):
    in_maps = build_in_maps(inputs)
    nc = _get_program()
    res = run_bass_kernel_spmd(nc, in_maps, core_ids=list(range(8)))
    outs = res.results

    y = np.empty((4, NTOK, C), np.float32)
    for core in range(8):
        b, half = core // 2, core % 2
        y[b, half * NOWN : (half + 1) * NOWN, :] = outs[core]["out"]
    return y


if __name__ == "__main__":
    prog = build_program()
    print("program built OK")



# revision 3
# speedup vs baseline: 1.2515x; 1.2515x over previous
"""Trainium2 Bass kernel for a dense transformer block (pre-LN, MHA + MLP).

Full inputs in, full outputs out. Sharding: 8 cores = (batch, seq-half).
Each core computes K/V over its batch element's full 1024 tokens and
Q/attention/MLP over its own 512 tokens (host permutes tokens so the core's
own half is always rows 0..511 — softmax over keys is permutation invariant).
No collectives needed.

Precision strategy:
  - Attention-side GEMMs (K, V, Q, attn@V, proj) run as fp8e4m3 DoubleRow
    matmuls (two 128-deep contraction slabs per instruction).  Softmax
    normalization + value averaging absorb the fp8 quantization noise.
    Weights are prescaled x16 on the host to sit in e4m3's normal range;
    the 1/16 (or 1/256) correction folds into the psum drain.
  - The MLP (fc1/fc2) stays bf16: fp8 there would blow the 2e-2 error gate.
  - Scores (64-deep contraction) stay bf16; DoubleRow needs 128-pairs.

V is computed token-major directly (stationary = hT chunk, moving = vw
slab) so no PE transposes are needed to build the [k-token, head-dim]
V layout for attn@V.

LayerNorm affine params are folded into the following matmul weights
on the host: (xhat*g + b) @ W == xhat @ (diag(g) W) + b @ W.
"""

import sys

sys.path.insert(0, "/opt/trn_rl_repo")

import numpy as np

import concourse.bass as bass
import concourse.bacc as bacc
import concourse.mybir as mybir
import concourse.tile as tile
from concourse.bass_utils import run_bass_kernel_spmd
from concourse.masks import make_identity

P = 128
C = 1024
HEADS = 16
DH = 64
HID = 4096
NTOK = 1024  # tokens per batch element (kv length)
NOWN = 512  # tokens owned by this core (q length)
SCALE = DH ** -0.5
EPS = 1e-5
WS = 16.0  # fp8 weight prescale
RWS = 1.0 / WS

F32 = mybir.dt.float32
BF16 = mybir.dt.bfloat16
FP8 = mybir.dt.float8e4
AF = mybir.ActivationFunctionType
OP = mybir.AluOpType
DR = mybir.MatmulPerfMode.DoubleRow

CT = C // P  # 8 column tiles of the model dim
TT = NTOK // P  # 8 token tiles (kv)
QT = NOWN // P  # 4 token tiles (own)
HT = HID // P  # 32 hidden tiles
VW = 128  # vh row stride (64 head dims + ones col + pad; dual-fp8
# ldweights requires the pair stride to be a multiple of 128)


def build_program():
    nc = bacc.Bacc("TRN2", target_bir_lowering=False)

    io = {}
    io["x"] = nc.dram_tensor("x", (NTOK, C), F32, kind="ExternalInput")
    io["qw"] = nc.dram_tensor("qw", (C, C), FP8, kind="ExternalInput")
    io["kw"] = nc.dram_tensor("kw", (C, C), FP8, kind="ExternalInput")
    io["vw"] = nc.dram_tensor("vw", (C, C), FP8, kind="ExternalInput")
    io["pw"] = nc.dram_tensor("pw", (C, C), FP8, kind="ExternalInput")
    io["f1w"] = nc.dram_tensor("f1w", (C, HID), BF16, kind="ExternalInput")
    io["f2w"] = nc.dram_tensor("f2w", (HID, C), BF16, kind="ExternalInput")
    # biases pre-transposed on host into [128, n] per-partition layout
    io["qbt"] = nc.dram_tensor("qbt", (P, CT), F32, kind="ExternalInput")
    io["kbt"] = nc.dram_tensor("kbt", (P, CT), F32, kind="ExternalInput")
    io["f1bt"] = nc.dram_tensor("f1bt", (P, HT), F32, kind="ExternalInput")
    io["vb"] = nc.dram_tensor("vb", (C,), F32, kind="ExternalInput")
    io["pb"] = nc.dram_tensor("pb", (C,), F32, kind="ExternalInput")
    io["f2b"] = nc.dram_tensor("f2b", (C,), F32, kind="ExternalInput")
    io["out"] = nc.dram_tensor("out", (NOWN, C), F32, kind="ExternalOutput")

    with tile.TileContext(nc) as tc:
        _emit(nc, tc, io)
    nc.compile()
    return nc


def _emit(nc, tc, io):
    x_d, out_d = io["x"], io["out"]

    with (
        tc.tile_pool(name="consts", bufs=1) as consts,
        tc.tile_pool(name="persist", bufs=1) as persist,
    ):
        # ---- constants ----
        ident_f32 = consts.tile([P, P], F32)
        make_identity(nc, ident_f32)
        ident = consts.tile([P, P], BF16)
        nc.vector.tensor_copy(out=ident, in_=ident_f32)
        eps_tile = consts.tile([P, 1], F32)
        nc.vector.memset(eps_tile, EPS)
        qbT = consts.tile([P, CT], F32)
        nc.sync.dma_start(qbT, io["qbt"][:, :])
        kbT = consts.tile([P, CT], F32)
        nc.sync.dma_start(kbT, io["kbt"][:, :])
        f1bT = consts.tile([P, HT], F32)
        nc.sync.dma_start(f1bT, io["f1bt"][:, :])

        def bcast_const(src_d, n):
            t = consts.tile([P, n], F32)
            src = bass.AP(tensor=src_d, offset=0, ap=[[0, P], [1, n]])
            nc.sync.dma_start(t, src)
            return t

        vb_bc = bcast_const(io["vb"], C)
        pb_bc = bcast_const(io["pb"], C)
        f2b_bc = bcast_const(io["f2b"], C)

        # own x tiles (fp32, kept for the residual), one tile per token tile
        x_own = []
        for t in range(QT):
            xo = persist.tile([P, C], F32, tag=f"xo{t}", name=f"xo{t}")
            nc.sync.dma_start(xo, x_d[t * P : (t + 1) * P, :])
            x_own.append(xo)
        x2 = [
            persist.tile([P, C], F32, tag=f"x2_{t}", name=f"x2_{t}")
            for t in range(QT)
        ]

        def layernorm_tile(temps, xt, out_dtype=BF16):
            """xt: [128, C] fp32 -> normalized tile [128, C] (out_dtype)."""
            stats = temps.tile([P, 2, 6], F32, tag="ln_stats", name="st")
            for sg in range(2):
                nc.vector.bn_stats(
                    out=stats[:, sg, :], in_=xt[:, sg * 512 : (sg + 1) * 512]
                )
            mv = temps.tile([P, 2], F32, tag="ln_mv", name="mv")
            nc.vector.bn_aggr(out=mv[:], in_=stats[:])
            rstd = temps.tile([P, 1], F32, tag="ln_rstd", name="rstd")
            nc.scalar.activation(
                out=rstd, in_=mv[:, 1:2], func=AF.Sqrt, bias=eps_tile, scale=1.0
            )
            nc.vector.reciprocal(out=rstd, in_=rstd)
            nmr = temps.tile([P, 1], F32, tag="ln_nmr", name="nmr")
            nc.vector.tensor_tensor(nmr, mv[:, 0:1], rstd, OP.mult)
            nc.vector.tensor_scalar_mul(nmr, nmr, -1.0)
            h = temps.tile([P, C], out_dtype, tag="ln_h", name="h")
            nc.vector.tensor_scalar(
                out=h,
                in0=xt,
                scalar1=rstd,
                scalar2=nmr,
                op0=OP.mult,
                op1=OP.add,
            )
            return h

        # ---- persistent attention-phase SBUF ----
        # hT[(j, t2)]: [128, 2, 512] fp8, c-slabs (2j, 2j+1), token half t2
        hT = {
            (j, t2): persist.tile(
                [P, 2, NOWN], FP8, tag=f"hT{j}_{t2}", name=f"hT{j}_{t2}"
            )
            for j in range(4)
            for t2 in range(2)
        }
        kT = {
            (ft, t2): persist.tile(
                [P, NOWN], BF16, tag=f"kT{ft}_{t2}", name=f"kT{ft}_{t2}"
            )
            for ft in range(CT)
            for t2 in range(2)
        }
        qT = [
            persist.tile([P, NOWN], BF16, tag=f"qT{ft}", name=f"qT{ft}")
            for ft in range(CT)
        ]
        # vh[h]: [k-token-part, k-tile, 64 dims + ones col] fp8
        vh = [
            persist.tile([P, TT, VW], FP8, tag=f"vh{h}", name=f"vh{h}")
            for h in range(HEADS)
        ]
        # oT pairs for proj DoubleRow: [feat-part, slab-pair, tok] fp8
        oT = [
            persist.tile([P, 2, NOWN], FP8, tag=f"oT{j}", name=f"oT{j}")
            for j in range(4)
        ]
        for h in range(HEADS):
            nc.vector.memset(vh[h][:, :, DH : DH + 1], 1.0)

        # fp8 weight slabs, all resident (3 MB total)
        kslab = [
            persist.tile([P, CT, P], FP8, tag=f"ks{ft}", name=f"ks{ft}")
            for ft in range(CT)
        ]
        qslab = [
            persist.tile([P, CT, P], FP8, tag=f"qs{ft}", name=f"qs{ft}")
            for ft in range(CT)
        ]
        vslab = [
            persist.tile([P, CT, NOWN], FP8, tag=f"vs{ns}", name=f"vs{ns}")
            for ns in range(2)
        ]
        for ft in range(CT):
            nc.sync.dma_start(
                kslab[ft],
                io["kw"][:, ft * P : (ft + 1) * P].rearrange(
                    "(c p) f -> p c f", p=P
                ),
            )
            nc.sync.dma_start(
                qslab[ft],
                io["qw"][:, ft * P : (ft + 1) * P].rearrange(
                    "(c p) f -> p c f", p=P
                ),
            )
        for ns in range(2):
            nc.sync.dma_start(
                vslab[ns],
                io["vw"][:, ns * NOWN : (ns + 1) * NOWN].rearrange(
                    "(c p) n -> p c n", p=P
                ),
            )

        # ================= Phase 1: LN1 -> hT (fp8, paired slabs) ========
        with (
            tc.tile_pool(name="ln1", bufs=3) as ln1,
            tc.tile_pool(name="xtmp", bufs=3) as xtmp,
            tc.tile_pool(name="tr1", bufs=2, space="PSUM") as tr1,
        ):
            for t in range(TT):
                if t < QT:
                    xt = x_own[t]
                else:
                    xt = xtmp.tile([P, C], F32, tag="xt", name="xt")
                    nc.sync.dma_start(xt, x_d[t * P : (t + 1) * P, :])
                h = layernorm_tile(ln1, xt)
                t2, tb = t // QT, t % QT
                for c in range(CT):
                    ps = tr1.tile([P, P], BF16, tag="tr", name="tr")
                    nc.tensor.transpose(
                        ps, h[:, c * P : (c + 1) * P], ident
                    )
                    nc.any.tensor_copy(
                        out=hT[(c // 2, t2)][
                            :, c % 2, tb * P : (tb + 1) * P
                        ],
                        in_=ps,
                    )

        # ============ Phase 2+3: V sweep, then per-ft K/Q + attention ====
        with (
            tc.tile_pool(name="qkv_ps", bufs=2, space="PSUM") as qkv_ps,
            tc.tile_pool(name="st_ps", bufs=2, space="PSUM") as st_ps,
            tc.tile_pool(name="ot_ps", bufs=2, space="PSUM") as ot_ps,
            tc.tile_pool(name="vt_sb", bufs=3) as vt_sb,
            tc.tile_pool(name="p_sb", bufs=2) as p_pool,
        ):
            # ---- V: token-major, vh[h][:, c, :64] = (h @ vw)/16 + vb ----
            for c in range(TT):
                j2, tb = c // QT, c % QT  # t2 half and tile within half
                for ns in range(2):
                    ps = qkv_ps.tile([P, NOWN], F32, tag="kvps", name="vps")
                    for j in range(4):
                        nc.tensor.matmul(
                            ps,
                            lhsT=hT[(j, j2)][:, :, tb * P : (tb + 1) * P],
                            rhs=vslab[ns][:, 2 * j : 2 * j + 2, :],
                            start=(j == 0),
                            stop=(j == 3),
                            perf_mode=DR,
                        )
                    vt = vt_sb.tile([P, NOWN], BF16, tag="vt", name="vt")
                    nc.vector.tensor_scalar(
                        out=vt, in0=ps, scalar1=RWS, scalar2=None, op0=OP.mult
                    )
                    nc.vector.tensor_tensor(
                        vt, vt, vb_bc[:, ns * NOWN : (ns + 1) * NOWN], OP.add
                    )
                    for hh in range(CT):
                        h_idx = ns * 8 + hh
                        nc.any.tensor_copy(
                            out=vh[h_idx][:, c, :DH],
                            in_=vt[:, hh * DH : (hh + 1) * DH],
                        )

            # ---- per-ft: K (both halves), Q, then heads 2ft, 2ft+1 ----
            for ft in range(CT):
                for t2 in range(2):
                    ps = qkv_ps.tile([P, NOWN], F32, tag="kvps", name="kps")
                    for j in range(4):
                        nc.tensor.matmul(
                            ps,
                            lhsT=kslab[ft][:, 2 * j : 2 * j + 2, :],
                            rhs=hT[(j, t2)],
                            start=(j == 0),
                            stop=(j == 3),
                            perf_mode=DR,
                        )
                    nc.vector.tensor_scalar(
                        out=kT[(ft, t2)],
                        in0=ps,
                        scalar1=RWS,
                        scalar2=kbT[:, ft : ft + 1],
                        op0=OP.mult,
                        op1=OP.add,
                    )
                ps = qkv_ps.tile([P, NOWN], F32, tag="kvps", name="qps")
                for j in range(4):
                    nc.tensor.matmul(
                        ps,
                        lhsT=qslab[ft][:, 2 * j : 2 * j + 2, :],
                        rhs=hT[(j, 0)],
                        start=(j == 0),
                        stop=(j == 3),
                        perf_mode=DR,
                    )
                nc.vector.tensor_scalar(
                    out=qT[ft],
                    in0=ps,
                    scalar1=RWS,
                    scalar2=qbT[:, ft : ft + 1],
                    op0=OP.mult,
                    op1=OP.add,
                )

                for hh in range(2):
                    h_idx = 2 * ft + hh
                    prow = hh * DH
                    p_sb = p_pool.tile(
                        [P, TT, NOWN], FP8, tag="p", name="p"
                    )
                    for g in range(4):  # pairs of k-tiles
                        stg = st_ps.tile(
                            [P, 2, NOWN], F32, tag="st", name="st"
                        )
                        for i in range(2):
                            c = 2 * g + i
                            nc.tensor.matmul(
                                stg[:, i, :],
                                lhsT=kT[(ft, c // 4)][
                                    prow : prow + DH,
                                    (c % 4) * P : (c % 4 + 1) * P,
                                ],
                                rhs=qT[ft][prow : prow + DH, :],
                                start=True,
                                stop=True,
                            )
                        nc.scalar.activation(
                            out=p_sb[:, 2 * g : 2 * g + 2, :],
                            in_=stg,
                            func=AF.Exp,
                            scale=SCALE,
                        )
                    ot = ot_ps.tile([P, NOWN], F32, tag="ot", name="ot")
                    for j in range(4):
                        nc.tensor.matmul(
                            ot[: DH + 1, :],
                            lhsT=vh[h_idx][:, 2 * j : 2 * j + 2, : DH + 1],
                            rhs=p_sb[:, 2 * j : 2 * j + 2, :],
                            start=(j == 0),
                            stop=(j == 3),
                            perf_mode=DR,
                        )
                    # softmax denominator in row DH (ones column of vh);
                    # oT = 16 * o / den  (x16 = fp8 range for proj)
                    rs = vt_sb.tile([1, NOWN], F32, tag="rs", name="rs")
                    nc.vector.reciprocal(out=rs, in_=ot[DH : DH + 1, :])
                    nc.vector.tensor_scalar_mul(rs, rs, WS)
                    rsb = vt_sb.tile([DH, NOWN], F32, tag="rsb", name="rsb")
                    nc.gpsimd.partition_broadcast(rsb, rs)
                    nc.vector.tensor_tensor(
                        oT[ft // 2][prow : prow + DH, ft % 2, :],
                        ot[:DH, :],
                        rsb,
                        OP.mult,
                    )

        # ======== Phase 4+5: proj + residual -> x2, LN2 -> h2T ===========
        h2T = [
            persist.tile([P, NOWN], BF16, tag=f"h2T{c}", name=f"h2T{c}")
            for c in range(CT)
        ]
        with (
            tc.tile_pool(name="proj_ps", bufs=1, space="PSUM") as proj_ps,
            tc.tile_pool(name="tr2", bufs=2, space="PSUM") as tr2,
            tc.tile_pool(name="pwc", bufs=2) as pwc,
            tc.tile_pool(name="ln2", bufs=2) as ln2,
        ):
            for ns in range(2):
                nsl = slice(ns * NOWN, (ns + 1) * NOWN)
                pslab = pwc.tile([P, CT, NOWN], FP8, tag="pw", name="pw")
                nc.sync.dma_start(
                    pslab,
                    io["pw"][:, nsl].rearrange("(c p) n -> p c n", p=P),
                )
                pss = [
                    proj_ps.tile([P, NOWN], F32, tag=f"pps{tq}", name=f"pp{tq}")
                    for tq in range(QT)
                ]
                for j2 in range(4):
                    for tq in range(QT):
                        nc.tensor.matmul(
                            pss[tq],
                            lhsT=oT[j2][:, :, tq * P : (tq + 1) * P],
                            rhs=pslab[:, 2 * j2 : 2 * j2 + 2, :],
                            start=(j2 == 0),
                            stop=(j2 == 3),
                            perf_mode=DR,
                        )
                for tq in range(QT):
                    # x2 = psum/256 + pb + x_own
                    nc.vector.tensor_scalar_mul(
                        pss[tq], pss[tq], 1.0 / (WS * WS)
                    )
                    nc.vector.tensor_add(pss[tq], pss[tq], pb_bc[:, nsl])
                    nc.vector.tensor_add(
                        x2[tq][:, nsl], pss[tq], x_own[tq][:, nsl]
                    )
            for t in range(QT):
                h = layernorm_tile(ln2, x2[t])
                for c in range(CT):
                    ps = tr2.tile([P, P], BF16, tag="tr", name="tr")
                    nc.tensor.transpose(ps, h[:, c * P : (c + 1) * P], ident)
                    nc.any.tensor_copy(
                        out=h2T[c][:, t * P : (t + 1) * P], in_=ps
                    )

        # ================= Phase 6: FC1 + gelu -> actT (bf16) ============
        actT = [
            persist.tile([P, NOWN], BF16, tag=f"actT{hf}", name=f"actT{hf}")
            for hf in range(HT)
        ]
        with (
            tc.tile_pool(name="f1c", bufs=4) as f1c,
            tc.tile_pool(name="f1_ps", bufs=4, space="PSUM") as f1_ps,
        ):
            for hf in range(HT):
                ps = f1_ps.tile([P, NOWN], F32, tag="f1ps", name="f1ps")
                slab = f1c.tile([P, CT, P], BF16, tag="f1w", name="f1slab")
                nc.sync.dma_start(
                    slab,
                    io["f1w"][:, hf * P : (hf + 1) * P].rearrange(
                        "(c p) f -> p c f", p=P
                    ),
                )
                for c in range(CT):
                    nc.tensor.matmul(
                        ps,
                        lhsT=slab[:, c, :],
                        rhs=h2T[c],
                        start=(c == 0),
                        stop=(c == CT - 1),
                    )
                nc.scalar.activation(
                    out=actT[hf],
                    in_=ps,
                    func=AF.Gelu,
                    bias=f1bT[:, hf : hf + 1],
                    scale=1.0,
                )

        # ================= Phase 7: FC2 + residual -> out ================
        with (
            tc.tile_pool(name="f2_ps", bufs=4, space="PSUM") as f2_ps,
            tc.tile_pool(name="f2c", bufs=1) as f2c,
            tc.tile_pool(name="out_sb", bufs=2) as out_pool,
        ):
            for ns in range(2):
                nsl = slice(ns * NOWN, (ns + 1) * NOWN)
                groups = []
                for g in range(8):
                    gw = f2c.tile(
                        [P, 4, NOWN], BF16, tag=f"g{g}", name=f"f2g{g}_{ns}"
                    )
                    nc.sync.dma_start(
                        gw,
                        io["f2w"][g * NOWN : (g + 1) * NOWN, nsl].rearrange(
                            "(o p) n -> p o n", p=P
                        ),
                    )
                    groups.append(gw)
                for tq in range(QT):
                    ps = f2_ps.tile([P, NOWN], F32, tag="f2ps", name="f2ps")
                    for hc in range(HT):
                        nc.tensor.matmul(
                            ps,
                            lhsT=actT[hc][:, tq * P : (tq + 1) * P],
                            rhs=groups[hc // 4][:, hc % 4, :],
                            start=(hc == 0),
                            stop=(hc == HT - 1),
                        )
                    ot2 = out_pool.tile([P, NOWN], F32, tag="out_t", name="o")
                    nc.vector.tensor_add(ps, ps, f2b_bc[:, nsl])
                    nc.vector.tensor_add(ot2, ps, x2[tq][:, nsl])
                    nc.sync.dma_start(out_d[tq * P : (tq + 1) * P, nsl], ot2)


_PROGRAM = None


def _get_program():
    global _PROGRAM
    if _PROGRAM is None:
        _PROGRAM = build_program()
    return _PROGRAM


def build_in_maps(inputs):
    import ml_dtypes

    E4 = ml_dtypes.float8_e4m3

    x = np.asarray(inputs["x"], np.float32)  # [4, 1024, 1024]
    ln1_g = np.asarray(inputs["ln1_g"], np.float64)
    ln1_b = np.asarray(inputs["ln1_b"], np.float64)
    ln2_g = np.asarray(inputs["ln2_g"], np.float64)
    ln2_b = np.asarray(inputs["ln2_b"], np.float64)
    qkv_w = np.asarray(inputs["qkv_w"], np.float64)
    qkv_b = np.asarray(inputs["qkv_b"], np.float64)
    proj_w = np.asarray(inputs["proj_w"], np.float64)
    proj_b = np.asarray(inputs["proj_b"], np.float32)
    fc1_w = np.asarray(inputs["fc1_w"], np.float64)
    fc1_b = np.asarray(inputs["fc1_b"], np.float64)
    fc2_w = np.asarray(inputs["fc2_w"], np.float64)
    fc2_b = np.asarray(inputs["fc2_b"], np.float32)

    # Fold LN affine into the following matmul:
    #   (xhat*g + b) @ W == xhat @ (diag(g) W) + b @ W
    qkv_w_f = ln1_g[:, None] * qkv_w
    qkv_b_f = (qkv_b + ln1_b @ qkv_w).astype(np.float32)
    f1w_f = ln2_g[:, None] * fc1_w
    f1b_f = (fc1_b + ln2_b @ fc1_w).astype(np.float32)

    qw8 = (qkv_w_f[:, :C] * WS).astype(np.float32).astype(E4)
    kw8 = (qkv_w_f[:, C : 2 * C] * WS).astype(np.float32).astype(E4)
    vw8 = (qkv_w_f[:, 2 * C :] * WS).astype(np.float32).astype(E4)
    pw8 = (proj_w * WS).astype(np.float32).astype(E4)
    f1w16 = f1w_f.astype(ml_dtypes.bfloat16)
    f2w16 = fc2_w.astype(ml_dtypes.bfloat16)

    def tbias(b):  # [n*128] -> [128, n] per-partition layout
        return np.ascontiguousarray(b.reshape(-1, P).T)

    common = dict(
        qw=np.ascontiguousarray(qw8),
        kw=np.ascontiguousarray(kw8),
        vw=np.ascontiguousarray(vw8),
        pw=np.ascontiguousarray(pw8),
        f1w=np.ascontiguousarray(f1w16),
        f2w=np.ascontiguousarray(f2w16),
        qbt=tbias(qkv_b_f[:C]),
        kbt=tbias(qkv_b_f[C : 2 * C]),
        f1bt=tbias(f1b_f),
        vb=np.ascontiguousarray(qkv_b_f[2 * C :]),
        pb=proj_b,
        f2b=fc2_b,
    )
    in_maps = []
    for core in range(8):
        b, half = core // 2, core % 2
        own = x[b, half * NOWN : (half + 1) * NOWN, :]
        other = x[b, (1 - half) * NOWN : (2 - half) * NOWN, :]
        xp = np.ascontiguousarray(np.concatenate([own, other], axis=0))
        in_maps.append({**common, "x": xp})
    return in_maps


def kernel(**inputs):
    in_maps = build_in_maps(inputs)
    nc = _get_program()
    res = run_bass_kernel_spmd(nc, in_maps, core_ids=list(range(8)))
    outs = res.results

    y = np.empty((4, NTOK, C), np.float32)
    for core in range(8):
        b, half = core // 2, core % 2
        y[b, half * NOWN : (half + 1) * NOWN, :] = outs[core]["out"]
    return y


if __name__ == "__main__":
    prog = build_program()
    print("program built OK")
